# revision 1
# baseline (speedup 1.0000x reference)
"""Trainium2 Bass kernel: clustered-topic cosine hinge loss (nn_CL_88399016886706).

reference:
    sim   = cosine_similarity(x, x)                         # [8192, 8192]
    mask  = (cid_i == cid_j) & (i < j)
    contrib = where(sim > 0.5, relu(1 - sim), relu(sim))
    out   = sum(where(mask, contrib, 0))                    # fp32 scalar

Algorithm used here (algebraically identical):
  * contrib == relu(0.5 - |sim - 0.5|) == 0.5 - min(|sim - 0.5|, 0.5)
    (continuous everywhere, including at sim == 0.5).
  * Stable-sort rows by cluster id.  Same-cluster pairs keep their relative
    order, so "i < j in original index" == "i' < j' in sorted index" for every
    masked pair.  After sorting, each cluster is a contiguous run of at most
    n_max rows, so every masked pair lives in a band j' - i' < n_max.
  * Each core owns 1024 sorted rows = 8 row-blocks of 128.  Per block it
    computes a [128, W] band of the Gram matrix of RAW x (W >= n_max + 127),
    so the matmuls start as soon as the DMA lands.  Only ~W/8192 of the full
    similarity matrix is ever computed.
  * Row norms^2 come from an ones[128,128]^T @ x^2 matmul (every partition
    gets the full vector), inv = sqrt(1/n2) in fp32; the per-partition
    layout inv_p is gathered by strided SBUF->SBUF DMAs (rows identical).
  * per-tile:  m = G * inv_j  (DVE, reads PSUM, free-dim scale);
    u = |inv_i * m - 0.5|    (ACT, per-partition scale + bias);
    sum(min(u, 0.5) * eq)    (DVE scalar_tensor_tensor with accum -> sm),
    where eq = (cid_col == cid_row) & (band col > band row) was prepared
    during the input DMA.  Since masked pairs contribute 0.5 - min(u, 0.5)
    each, total = 0.5 * (#same-cluster ordered pairs) - sum(sm); the pair
    count is pure cluster-id bookkeeping done on the host.

The 8 cores each take 8 consecutive sorted row-blocks and the column window
[first_row, first_row + 1024 + W - 128); the scalar partials are summed on the
host (the "all-reduce" of the sharding hint, done after gather).
"""

import math

import numpy as np
import ml_dtypes

P = 128
N_CORES = 8

_BF16 = ml_dtypes.bfloat16
_FP8 = ml_dtypes.float8_e4m3

# fp8e4m3 for the x^T payload halves the dominant DMA; squares of fp8 are
# exact in bf16 and the Gram accumulates in fp32.  Measured end-to-end error
# stays ~1e-3 of the fp64 reference (threshold 2e-2).
USE_FP8 = False

_prog_cache = {}


_MAX_SYNC_WAITS = 1  # walrus in this container rejects >2 sync waits per inst


def _split_excess_sync_waits(nc, limit=_MAX_SYNC_WAITS):
    """Move excess per-instruction semaphore waits onto injected nops.

    The walrus build shipped here rejects instructions carrying more than
    `limit` sync-wait commands ("Too many sync wait commands"), which the
    TileContext tail drain (one wait per active semaphore) exceeds.  Engines
    execute their stream in order, so hoisting the first waits onto same-
    engine nops immediately before the instruction is semantically identical.
    """
    import concourse.mybir as mybir

    n = 0
    for bb in nc.main_func.blocks:
        out = []
        for inst in bb.instructions:
            si = getattr(inst, "sync_info", None)
            waits = list(si.on_wait) if si is not None and si.on_wait else []
            if len(waits) > limit:
                excess, keep = waits[:-limit], waits[-limit:]
                for j in range(0, len(excess), limit):
                    nop = mybir.InstNoOp(
                        name=f"wsplit-{inst.name}-{j}", ins=[], outs=[])
                    nop.engine = inst.engine
                    nop.sync_info = mybir.SyncInfo(
                        on_wait=excess[j:j + limit], on_update=[])
                    out.append(nop)
                    n += 1
                si.on_wait = keep
            out.append(inst)
        bb.instructions[:] = out
    return n


def _build_program(D, rows_per_core, W, W_in, use_fp8=USE_FP8,
                   split_waits=True):
    import concourse.bass as bass
    import concourse.mybir as mybir
    import concourse.tile as tile
    from contextlib import ExitStack

    fp32 = mybir.dt.float32
    bf16 = mybir.dt.bfloat16
    xdt = mybir.dt.float8e4 if use_fp8 else bf16
    AO = mybir.AluOpType
    AF = mybir.ActivationFunctionType

    n_chunks = D // P            # contraction chunks (embed dim)
    n_blocks = rows_per_core // P

    # norm-reduction slices of the column window (PSUM bank = 512 fp32):
    # matmul outputs must stay inside one bank, so slices are 512-aligned
    n_slc = math.ceil(W_in / 512)
    slc_off = [s * 512 for s in range(n_slc)]
    slc_sizes = [min(512, W_in - o) for o in slc_off]

    nc = bass.Bass("TRN2", target_bir_lowering=False, debug=False)

    xt_d = nc.dram_tensor("xt", [D, W_in], xdt, kind="ExternalInput").ap()
    cidb_d = nc.dram_tensor("cidb", [P, W_in], bf16, kind="ExternalInput").ap()
    cidp_d = nc.dram_tensor("cidp", [P, n_blocks], fp32, kind="ExternalInput").ap()
    trin_d = nc.dram_tensor("trin", [P, W], fp32, kind="ExternalInput").ap()
    ones128_d = nc.dram_tensor("ones128", [P, P], bf16, kind="ExternalInput").ap()
    n_sm = rows_per_core // P
    out_d = nc.dram_tensor("out_sums", [P, n_sm], fp32,
                           kind="ExternalOutput").ap()

    with tile.TileContext(nc) as tc, ExitStack() as ctx:
        const = ctx.enter_context(tc.tile_pool(name="const", bufs=1))
        xp = ctx.enter_context(tc.tile_pool(name="xp", bufs=1))
        sqp = ctx.enter_context(tc.tile_pool(name="sqp", bufs=8))
        wp = ctx.enter_context(tc.tile_pool(name="wp", bufs=3))
        pp = ctx.enter_context(tc.tile_pool(name="pp", bufs=1, space="PSUM"))
        pgp = ctx.enter_context(tc.tile_pool(name="pgp", bufs=3, space="PSUM"))

        # x^T (embed on partitions, sorted topic window on free dim) — the
        # bulk transfer goes first on the HWDGE queues; constants ride SWDGE.
        xts = [xp.tile([P, W_in], xdt, tag=f"xt{k}", name=f"xts{k}")
               for k in range(n_chunks)]
        xt_r = xt_d.rearrange("(k p) w -> p k w", p=P)
        split_k = max(0, n_chunks - 2) if W_in > 512 else n_chunks
        # two DMA queues in parallel: sync carries the ACT-destined chunks
        # and the split tail chunks; gpsimd carries k4/k5 (DVE-destined)
        # between the constants
        pool_chunks = [k for k in (4, 5) if k < split_k]
        for k in range(split_k):
            if k not in pool_chunks:
                nc.sync.dma_start(xts[k], xt_r[:, k, :])
        cidb = const.tile([P, W_in], bf16)
        nc.gpsimd.dma_start(cidb, cidb_d)
        ones128 = const.tile([P, P], bf16)
        nc.gpsimd.dma_start(ones128, ones128_d)
        cidp = const.tile([P, n_blocks], fp32)
        nc.gpsimd.dma_start(cidp, cidp_d)
        trin = const.tile([P, W], fp32)
        nc.gpsimd.dma_start(trin, trin_d)
        for k in pool_chunks:
            nc.gpsimd.dma_start(xts[k], xt_r[:, k, :])
        for k in range(split_k, n_chunks):
            # the tail chain is gated on the last chunks: land their
            # first 512 columns (slice 0) early
            nc.sync.dma_start(xts[k][:, 0:512], xt_r[:, k, 0:512])
            nc.sync.dma_start(xts[k][:, 512:], xt_r[:, k, 512:])
        halfneg = const.tile([P, 1], fp32)
        nc.vector.memset(halfneg, -0.5)
        wsrc = const.tile([P, 1], fp32)
        nc.vector.memset(wsrc, 1.0)

        # warm the ACT tables (first activation otherwise pays the cold
        # table load on the critical path)
        wdummy = const.tile([P, 1], fp32)
        nc.scalar.activation(wdummy, wsrc, AF.Square)
        nc.scalar.activation(wdummy, wsrc, AF.Sqrt)
        nc.scalar.activation(wdummy, wsrc, AF.Abs, bias=halfneg)

        # ---- masks depend only on cluster ids: compute during the x DMA.
        # eq[b] = (cid_col == cid_row) (DVE), then & (band col > band row)
        # via the 0/1 trin pattern (GpSimd).  sum(eq) over the mask is pure
        # cluster-id bookkeeping and is counted on the host.
        sm = const.tile([P, n_sm], fp32)       # sum(min(u,.5)*eq) per pair
        eqall = const.tile([P, n_blocks, W], fp32)
        for b in range(n_blocks):
            c0 = b * P
            nc.vector.tensor_scalar(
                eqall[:, b, :], cidb[:, c0:c0 + W], cidp[:, b:b + 1], None,
                AO.is_equal)
        for b in range(n_blocks):
            nc.gpsimd.tensor_tensor(eqall[:, b, :], eqall[:, b, :], trin,
                                    AO.mult)

        # ---- norms^2 in two layouts, straight off the squares:
        #  * broadcast [128, W_in]: ones[128,128]^T @ sq  (inv_j, free dim)
        #  * partition [128, n_blocks]: sq-block^T @ ones  (inv_i, per row)
        # one 3-bank PSUM tile; each matmul output slice stays in one bank
        pn = [pp.tile([P, slc_sizes[s]], fp32, tag=f"pn{s}", name=f"pn{s}")
              for s in range(n_slc)]
        sq_last = None
        last_s0_mm = None
        for k in range(n_chunks):
            sq = sqp.tile([P, W_in], bf16, tag="sq", name=f"sq{k}")
            last = k == n_chunks - 1
            if k < 4:
                nc.scalar.activation(sq, xts[k], AF.Square)
            elif k < split_k:
                nc.vector.tensor_tensor(sq, xts[k], xts[k], AO.mult)
            elif not last:
                nc.vector.tensor_tensor(sq[:, 0:512], xts[k][:, 0:512],
                                        xts[k][:, 0:512], AO.mult)
                nc.vector.tensor_tensor(sq[:, 512:], xts[k][:, 512:],
                                        xts[k][:, 512:], AO.mult)
            else:
                # last chunk: slice-0 half only; second half is deferred
                # until after recip/sqrt of slice 0 is queued
                sq_last = sq
                nc.vector.tensor_tensor(sq[:, 0:512], xts[k][:, 0:512],
                                        xts[k][:, 0:512], AO.mult)
            for s in range(n_slc):
                if last and sq_last is not None and s > 0:
                    continue
                mm = nc.tensor.matmul(
                    pn[s], lhsT=ones128,
                    rhs=sq[:, slc_off[s]:slc_off[s] + slc_sizes[s]],
                    start=(k == 0), stop=(k == n_chunks - 1))
                if last and s == 0:
                    last_s0_mm = mm

        # ---- inv_norm = sqrt(1/norms^2) broadcast layout (fp32); slice 0
        # first (it gates the whole mask/reduce tail), then the deferred
        # second half of the last chunk, then the remaining slices ----
        inv2b = const.tile([P, W_in], fp32)
        inv_b = const.tile([P, W_in], fp32)
        sl0 = slice(slc_off[0], slc_off[0] + slc_sizes[0])
        nc.vector.reciprocal(inv2b[:, sl0], pn[0])
        nc.scalar.activation(inv_b[:, sl0], inv2b[:, sl0], AF.Sqrt)
        if sq_last is not None:
            nc.vector.tensor_tensor(sq_last[:, 512:], xts[n_chunks - 1][:, 512:],
                                    xts[n_chunks - 1][:, 512:], AO.mult)
            for s in range(1, n_slc):
                nc.tensor.matmul(
                    pn[s], lhsT=ones128,
                    rhs=sq_last[:, slc_off[s]:slc_off[s] + slc_sizes[s]],
                    start=False, stop=True)
        for s in range(1, n_slc):
            sl = slice(slc_off[s], slc_off[s] + slc_sizes[s])
            nc.vector.reciprocal(inv2b[:, sl], pn[s])
            nc.scalar.activation(inv_b[:, sl], inv2b[:, sl], AF.Sqrt)
        # per-partition inv layout inv_p[p, b] = inv[b*128 + p]: inv_b rows
        # are identical, so a strided SBUF->SBUF DMA from row 0 gathers it;
        # one DMA per 512-slice so each fires right after its sqrt.
        inv_p = const.tile([P, n_blocks], fp32)
        for b in range(n_blocks):
            c0 = b * P
            eng = nc.sync if b % 2 == 0 else nc.gpsimd
            eng.dma_start(inv_p[:, b:b + 1], inv_b[0:1, c0:c0 + P])

        # ---- banded Gram on RAW x (starts as soon as the DMA lands), then
        # sim = G * inv_i * inv_j folded into the mask/reduce tail ----
        from concourse.tile_rust import add_dep_helper
        for b in range(n_blocks):
            c0 = b * P
            pg = pgp.tile([P, W], fp32, tag="pg", name=f"pg{b}")
            for k in range(n_chunks):
                mm = nc.tensor.matmul(pg, lhsT=xts[k][:, c0:c0 + P],
                                      rhs=xts[k][:, c0:c0 + W],
                                      start=(k == 0), stop=(k == n_chunks - 1))
                if b == 0 and k == 0 and last_s0_mm is not None:
                    add_dep_helper(mm.ins, last_s0_mm.ins, sync=False,
                                   reason="norm slice-0 close precedes gram")
            m = wp.tile([P, W], fp32, tag="m", name=f"m{b}")
            nc.vector.tensor_tensor(m, pg, inv_b[:, c0:c0 + W], AO.mult)
            u = wp.tile([P, W], fp32, tag="u", name=f"u{b}")
            nc.scalar.activation(u, m, AF.Abs, scale=inv_p[:, b:b + 1],
                                 bias=halfneg)
            scr = wp.tile([P, W], fp32, tag="scr", name=f"scr{b}")
            nc.vector.scalar_tensor_tensor(
                scr, u, 0.5, eqall[:, b, :], AO.min, AO.mult,
                accum_out=sm[:, b:b + 1])

        # ship the [128, n_blocks] partial sums; the host finishes the
        # reduction as part of the gather
        nc.sync.dma_start(out_d, sm)

    if split_waits:  # needed for walrus compile; breaks CoreSim bookkeeping
        _split_excess_sync_waits(nc)
    return nc


def _prepare(topic_embeddings, cluster_ids):
    """Host-side sharding: sort by cluster, transpose, slice per core."""
    x = np.asarray(topic_embeddings, dtype=np.float32)
    cid = np.asarray(cluster_ids).astype(np.int64)
    K, D = x.shape
    assert K % N_CORES == 0 and D % P == 0
    rows_per_core = K // N_CORES
    n_blocks = rows_per_core // P

    perm = np.argsort(cid, kind="stable")
    xs = x[perm]
    cs = cid[perm]
    n_max = int(np.bincount(cid, minlength=1).max())
    W = int(math.ceil((n_max + P - 1) / 64)) * 64   # W >= n_max + 127
    if W < 192:
        W = 192
    assert W <= 512, f"cluster of size {n_max} too large for single-bank band"
    pad = W - P
    W_in = rows_per_core + pad

    xs_pad = np.ones((K + pad, D), np.float32)  # pad rows: norm>0, cid=-1
    xs_pad[:K] = xs
    cs_pad = np.full(K + pad, -1.0, np.float32)
    cs_pad[:K] = cs
    xT = np.ascontiguousarray(xs_pad.T)          # [D, K+pad]

    tri = np.arange(W)[None, :] > np.arange(P)[:, None]
    trin = tri.astype(np.float32)
    ones128 = np.ones((P, P), _BF16)

    in_maps = []
    for c in range(N_CORES):
        lo = c * rows_per_core
        xt_c = np.ascontiguousarray(xT[:, lo:lo + W_in]).astype(
            _FP8 if USE_FP8 else _BF16)
        cidw = cs_pad[lo:lo + W_in].astype(_BF16)
        cidb = np.ascontiguousarray(np.broadcast_to(cidw[None, :], (P, W_in)))
        cidp = np.ascontiguousarray(
            cs[lo:lo + rows_per_core].reshape(n_blocks, P).T.astype(np.float32))
        in_maps.append({
            "xt": xt_c, "cidb": cidb, "cidp": cidp, "trin": trin,
            "ones128": ones128,
        })
    return in_maps, (D, rows_per_core, W, W_in)


def run(topic_embeddings, cluster_ids, trace=False):
    from concourse.bass_utils import run_bass_kernel_spmd

    in_maps, key = _prepare(topic_embeddings, cluster_ids)
    key = key + (USE_FP8,)
    if key not in _prog_cache:
        _prog_cache[key] = _build_program(*key[:4], use_fp8=key[4])
    nc = _prog_cache[key]
    res = run_bass_kernel_spmd(nc, in_maps, core_ids=list(range(N_CORES)),
                               trace=trace)
    # Each masked pair contributes 0.5 - min(u, 0.5); the device returns
    # sum(min(u, 0.5)*eq) and the 0.5-per-pair term is the same-cluster
    # ordered-pair count, i.e. sum over clusters of C(n_c, 2).
    counts = np.bincount(np.asarray(cluster_ids).astype(np.int64))
    n_pairs = float((counts.astype(np.float64) *
                     (counts.astype(np.float64) - 1.0) / 2.0).sum())
    total = 0.5 * n_pairs
    for c in range(N_CORES):
        total -= float(np.asarray(res.results[c]["out_sums"],
                                  np.float64).sum())
    return np.float32(total), res


def kernel(topic_embeddings, cluster_ids):
    value, _ = run(topic_embeddings, cluster_ids, trace=False)
    return value



# revision 6
# speedup vs baseline: 1.7114x; 1.7114x over previous
"""Trainium2 Bass kernel: clustered-topic cosine hinge loss (nn_CL_88399016886706).

reference:
    sim   = cosine_similarity(x, x)                         # [8192, 8192]
    mask  = (cid_i == cid_j) & (i < j)
    contrib = where(sim > 0.5, relu(1 - sim), relu(sim))
    out   = sum(where(mask, contrib, 0))                    # fp32 scalar

Algorithm (algebraically identical):
  * contrib == 0.5 - min(|sim - 0.5|, 0.5) for every sim (continuous at the
    threshold), so the loss is pair-count bookkeeping minus a sum of
    min-abs terms over same-cluster pairs.
  * The 192 clusters are packed into 64 bins of EXACTLY 128 rows (an exact
    3-partition found by a randomized seed-and-pair search; BFD fallback
    pads with zero rows if no perfect packing exists).  Every same-cluster
    pair lives inside one bin, so only the 64 diagonal 128x128 blocks of
    the Gram matrix are ever computed: 8 bins per core.
  * Cluster membership is folded into the Gram matmul itself: each row
    vector is [x_i / ||x_i|| (1024 dims); 2*e_ord (16 dims)] where ord is
    the cluster's ordinal within its bin.  Then G' = sim + 4*same_cluster,
    and min(|G' - 4.5|, 0.5) equals min(|sim - 0.5|, 0.5) for same-cluster
    pairs and EXACTLY 0.5 for cross-cluster / padding entries (margin > 3).
    No cluster-id tensors, masks, or triangle patterns on the device.
  * fp8e4m3 inputs with DoubleRow matmuls (0.5 PE cycles per output column,
    256-deep contraction per instruction): 5 matmuls per bin.  Per-bin PSUM
    blocks are grouped [128, 512] so one Activation (|g - 4.5| -> fp16) and
    one DVE tensor-scalar (min 0.5, accumulated sum -> [128,1]) handle four
    bins each.  A single [128, 2] DMA returns the partial sums.
  * Host finishes: sum = 0.5*P - (S - 0.5*C - D)/2 where P/C are pair
    counts from cluster sizes and D is the exact diagonal term computed
    from the quantized vectors.
"""

import math

import numpy as np
import ml_dtypes

P = 128
N_CORES = 8
D_EMB = 1024
SIG = 16           # signature dims (max clusters per bin)
LAMBDA = 4.0       # sig one-hot value 2.0 -> same-cluster dot = 4
BIAS = -(LAMBDA + 0.5)

_FP8 = ml_dtypes.float8_e4m3

_prog_cache = {}

_MAX_SYNC_WAITS = 1  # walrus in this container rejects >2 sync waits per inst


def _split_excess_sync_waits(nc, limit=_MAX_SYNC_WAITS):
    """Move excess per-instruction semaphore waits onto injected nops.

    The walrus build shipped here rejects instructions carrying more than
    `limit` sync-wait commands ("Too many sync wait commands"), which the
    TileContext tail drain (one wait per active semaphore) exceeds.  Engines
    execute their stream in order, so hoisting the first waits onto same-
    engine nops immediately before the instruction is semantically identical.
    """
    import concourse.mybir as mybir

    n = 0
    for bb in nc.main_func.blocks:
        out = []
        for inst in bb.instructions:
            si = getattr(inst, "sync_info", None)
            waits = list(si.on_wait) if si is not None and si.on_wait else []
            if len(waits) > limit:
                excess, keep = waits[:-limit], waits[-limit:]
                for j in range(0, len(excess), limit):
                    nop = mybir.InstNoOp(
                        name=f"wsplit-{inst.name}-{j}", ins=[], outs=[])
                    nop.engine = inst.engine
                    nop.sync_info = mybir.SyncInfo(
                        on_wait=excess[j:j + limit], on_update=[])
                    out.append(nop)
                    n += 1
                si.on_wait = keep
            out.append(inst)
        bb.instructions[:] = out
    return n


# ---------------------------------------------------------------------------
# Bin packing: clusters -> bins of exactly 128 rows
# ---------------------------------------------------------------------------

def _pack_bins(sizes):
    """Pack cluster sizes into bins of capacity 128.

    Returns a list of bins (each a list of cluster ids).  Tries hard for an
    exact packing (every bin exactly 128 -> no padding); falls back to
    best-fit-decreasing.
    """
    n = len(sizes)
    total = int(np.sum(sizes))

    def bfd():
        order = np.argsort(-sizes, kind="stable")
        bins, rem = [], []
        for c in order:
            s = int(sizes[c])
            best = -1
            for i in range(len(bins)):
                if rem[i] >= s and (best < 0 or rem[i] < rem[best]):
                    best = i
            if best >= 0:
                bins[best].append(int(c))
                rem[best] -= s
            else:
                bins.append([int(c)])
                rem.append(P - s)
        return bins

    if total % P != 0 or np.max(sizes) > P:
        return bfd()
    nb = total // P

    def pair_solve(seed):
        """Seed bins with the nb largest clusters, fill each with an exact
        pair from the remainder."""
        rng = np.random.default_rng(seed)
        idx = np.argsort(-(sizes + rng.random(n) * 1e-6), kind="stable")
        big, small = list(idx[:nb]), list(idx[nb:])
        from collections import defaultdict
        avail = defaultdict(list)
        for c in small:
            avail[int(sizes[c])].append(int(c))
        bins, fails = [], []
        order = list(big)
        rng.shuffle(order)
        for a in order:
            r = P - int(sizes[a])
            xs = list(avail.keys())
            rng.shuffle(xs)
            found = None
            for x in xs:
                y = r - x
                if y in avail:
                    if x == y and len(avail[x]) < 2:
                        continue
                    found = (x, y)
                    break
            if found:
                x, y = found
                b = avail[x].pop()
                c = avail[y].pop()
                if not avail[x]:
                    del avail[x]
                if y in avail and not avail[y]:
                    del avail[y]
                bins.append([int(a), b, c])
            else:
                fails.append(int(a))
        left = [c for v in avail.values() for c in v] + fails
        return bins, left

    def partition_exact(items):
        """DFS: partition items into subsets each summing exactly 128."""
        items = sorted(items, key=lambda c: -sizes[c])
        m = len(items)
        if sum(int(sizes[c]) for c in items) % P != 0:
            return None
        used = [False] * m
        out = []
        calls = [0]

        def solve():
            calls[0] += 1
            if calls[0] > 200000:
                return False
            i0 = next((i for i in range(m) if not used[i]), None)
            if i0 is None:
                return True
            used[i0] = True
            cur = [items[i0]]

            def complete(start, cursum):
                if cursum == P:
                    out.append(cur[:])
                    if solve():
                        return True
                    out.pop()
                    return False
                for i in range(start, m):
                    if used[i] or cursum + sizes[items[i]] > P:
                        continue
                    if (i > start and sizes[items[i]] == sizes[items[i - 1]]
                            and not used[i - 1]):
                        continue
                    used[i] = True
                    cur.append(items[i])
                    if complete(i + 1, cursum + int(sizes[items[i]])):
                        return True
                    used[i] = False
                    cur.pop()
                return False

            if complete(i0 + 1, int(sizes[items[i0]])):
                return True
            used[i0] = False
            return False

        return out if solve() else None

    best_bins, best_left = None, None
    for seed in range(400):
        bins, left = pair_solve(seed)
        if best_bins is None or len(bins) > len(best_bins):
            best_bins, best_left = bins, left
        if not left and len(bins) == nb:
            return bins
    # repair: break a few bins, exhaustively re-partition with the leftovers
    bins, left = best_bins, best_left
    rng = np.random.default_rng(12345)
    for _ in range(300):
        if not left:
            break
        k = int(rng.integers(1, 5))
        k = min(k, len(bins))
        pick = set(rng.choice(len(bins), size=k, replace=False).tolist())
        pool = list(left)
        for i in pick:
            pool += bins[i]
        res = partition_exact(pool)
        if res is not None:
            bins = [b for i, b in enumerate(bins) if i not in pick] + res
            left = []
            break
    if left:
        return bfd()
    return bins


# ---------------------------------------------------------------------------
# Device program
# ---------------------------------------------------------------------------

def _build_program(n_bins_core, split_waits=True):
    import concourse.bass as bass
    import concourse.mybir as mybir
    import concourse.tile as tile
    from contextlib import ExitStack

    fp32 = mybir.dt.float32
    fp16 = mybir.dt.float16
    fp8 = mybir.dt.float8e4
    AF = mybir.ActivationFunctionType
    AO = mybir.AluOpType
    DR = mybir.MatmulPerfMode.DoubleRow

    C = n_bins_core * P          # columns (topics) per core
    n_dr = D_EMB // (2 * P)      # 4 DoubleRow chunks over the 1024 embed dims
    # tail groups of up to 4 bins (one [128, 4*128] PSUM tile = 1 bank each)
    groups = []
    b0 = 0
    while b0 < n_bins_core:
        gw = min(4, n_bins_core - b0)
        groups.append((b0, gw))
        b0 += gw
    n_g = len(groups)

    nc = bass.Bass("TRN2", target_bir_lowering=False, debug=False)

    xt_d = nc.dram_tensor("xt", [D_EMB, C], fp8, kind="ExternalInput").ap()
    sig_d = nc.dram_tensor("sig", [SIG, C], fp8, kind="ExternalInput").ap()
    out_d = nc.dram_tensor("out_sums", [P, n_g], fp32,
                           kind="ExternalOutput").ap()

    with tile.TileContext(nc) as tc, ExitStack() as ctx:
        const = ctx.enter_context(tc.tile_pool(name="const", bufs=1))
        xp = ctx.enter_context(tc.tile_pool(name="xp", bufs=1))
        wp = ctx.enter_context(tc.tile_pool(name="wp", bufs=1))
        pgp = ctx.enter_context(tc.tile_pool(name="pgp", bufs=n_g,
                                             space="PSUM"))

        # ---- input DMAs (SP queue): sig first (small -> PE starts early),
        # then the four DoubleRow x chunks.
        sigt = const.tile([SIG // 2, 2, C], fp8)
        sig_r = sig_d.rearrange("(two p) c -> p two c", p=SIG // 2)
        nc.sync.dma_start(sigt, sig_r)
        xts = [xp.tile([P, 2, C], fp8, tag=f"xt{k}", name=f"xts{k}")
               for k in range(n_dr)]
        xt_r = xt_d.rearrange("(k two p) c -> p k two c", p=P, two=2)
        for k in range(n_dr):
            nc.sync.dma_start(xts[k], xt_r[:, k, :, :])

        # ---- bias vector + warm the ACT Abs table off the critical path
        biasv = const.tile([P, 1], fp32)
        nc.vector.memset(biasv, BIAS)
        wsrc = const.tile([P, 1], fp32)
        nc.vector.memset(wsrc, 1.0)
        wdummy = const.tile([P, 1], fp32)
        nc.scalar.activation(wdummy, wsrc, AF.Abs, bias=biasv)

        # ---- per-group PSUM tiles; per-bin DoubleRow matmul accumulation
        pgs = [pgp.tile([P, gw * P], fp32, tag=f"pg{g}", name=f"pg{g}")
               for g, (_, gw) in enumerate(groups)]
        sm = const.tile([P, n_g], fp32)

        def bin_slices():
            for g, (b0, gw) in enumerate(groups):
                for j in range(gw):
                    b = b0 + j
                    yield g, b, pgs[g][:, j * P:(j + 1) * P], b * P

        # One accumulation group per PSUM bank (2KB zero region): the first
        # matmul into a bank carries start=True (lazy-zeroes the whole bank),
        # only the very last matmul into it carries stop=True.
        # sig matmuls open every bank as soon as sig lands.
        for g, b, pg, c0 in bin_slices():
            first_in_bank = b == groups[g][0]
            nc.tensor.matmul(pg, lhsT=sigt[:, :, c0:c0 + P],
                             rhs=sigt[:, :, c0:c0 + P],
                             start=first_in_bank, stop=False, perf_mode=DR)
        # x chunks, chunk-major so PE streams as DMAs land
        for k in range(n_dr):
            last = k == n_dr - 1
            for g, b, pg, c0 in bin_slices():
                last_in_bank = last and b == groups[g][0] + groups[g][1] - 1
                nc.tensor.matmul(pg, lhsT=xts[k][:, :, c0:c0 + P],
                                 rhs=xts[k][:, :, c0:c0 + P],
                                 start=False, stop=last_in_bank,
                                 perf_mode=DR)

        # ---- tail: u = |g - 4.5| (ACT, psum->sbuf fp16), then
        # min(u, 0.5) summed per partition (DVE, 4x mode) -> sm column
        for g, (b0, gw) in enumerate(groups):
            u = wp.tile([P, gw * P], fp16, tag=f"u{g}", name=f"u{g}")
            nc.scalar.activation(u, pgs[g], AF.Abs, bias=biasv)
            nc.vector.tensor_scalar(u, u, 0.5, 0.0, AO.min, AO.max,
                                    accum_out=sm[:, g:g + 1])

        nc.sync.dma_start(out_d, sm)

    if split_waits:  # needed for walrus compile; breaks CoreSim bookkeeping
        _split_excess_sync_waits(nc)
    return nc


# ---------------------------------------------------------------------------
# Host side
# ---------------------------------------------------------------------------

def _prepare(topic_embeddings, cluster_ids):
    x = np.asarray(topic_embeddings, dtype=np.float32)
    cid = np.asarray(cluster_ids).astype(np.int64)
    K, D = x.shape
    assert D == D_EMB

    sizes = np.bincount(cid)
    bins = _pack_bins(sizes)
    n_bins = len(bins)
    n_bins_core = math.ceil(n_bins / N_CORES)
    n_slots = n_bins_core * N_CORES          # bins incl. dummy all-pad bins

    # rows of each cluster in original order
    order = np.argsort(cid, kind="stable")
    starts = np.zeros(len(sizes) + 1, np.int64)
    np.cumsum(sizes, out=starts[1:])

    # row layout: bin-by-bin; per bin clusters consecutive
    perm = np.full(n_slots * P, -1, np.int64)     # padded row -> orig row
    sig_ord = np.zeros(n_slots * P, np.int64)     # within-bin cluster ordinal
    pos = 0
    for b, members in enumerate(bins):
        pos = b * P
        assert len(members) <= SIG
        for j, c in enumerate(members):
            rows = order[starts[c]:starts[c + 1]]
            perm[pos:pos + len(rows)] = rows
            sig_ord[pos:pos + len(rows)] = j
            pos += len(rows)
        assert pos <= (b + 1) * P

    # normalize + quantize
    xn = x / np.linalg.norm(x, axis=1, keepdims=True)
    q = np.zeros((n_slots * P, D), _FP8)
    real = perm >= 0
    q[real] = xn[perm[real]].astype(_FP8)
    sig = np.zeros((n_slots * P, SIG), _FP8)
    sig[real, sig_ord[real]] = _FP8(2.0)

    xT = np.ascontiguousarray(q.T)               # [1024, n_slots*128]
    sigT = np.ascontiguousarray(sig.T)           # [16,   n_slots*128]

    C = n_bins_core * P
    in_maps = []
    for c in range(N_CORES):
        lo = c * C
        in_maps.append({
            "xt": np.ascontiguousarray(xT[:, lo:lo + C]),
            "sig": np.ascontiguousarray(sigT[:, lo:lo + C]),
        })

    # ---- host-side constants ----
    sz = sizes.astype(np.float64)
    pairs_total = float((sz * (sz - 1) / 2).sum())          # P
    same_offdiag = 0.0                                      # ordered, per-bin
    for members in bins:
        for c in members:
            same_offdiag += sizes[c] * (sizes[c] - 1)
    cross_offdiag = n_slots * P * (P - 1) - same_offdiag    # C
    # exact diagonal term: G'_ii = ||q_i||^2 + 4 (or 0 for pad rows)
    qf = q.astype(np.float32)
    g_ii = (qf * qf).sum(axis=1, dtype=np.float64)
    g_ii[real] += LAMBDA
    diag = float(np.minimum(np.abs(g_ii + BIAS), 0.5).sum())  # D

    consts = (pairs_total, cross_offdiag, diag)
    return in_maps, n_bins_core, consts


def run(topic_embeddings, cluster_ids, trace=False):
    from concourse.bass_utils import run_bass_kernel_spmd

    in_maps, n_bins_core, (pairs_total, cross_offdiag, diag) = _prepare(
        topic_embeddings, cluster_ids)
    if n_bins_core not in _prog_cache:
        _prog_cache[n_bins_core] = _build_program(n_bins_core)
    nc = _prog_cache[n_bins_core]
    res = run_bass_kernel_spmd(nc, in_maps, core_ids=list(range(N_CORES)),
                               trace=trace)
    s_total = 0.0
    for c in range(N_CORES):
        s_total += float(np.asarray(res.results[c]["out_sums"],
                                    np.float64).sum())
    m = (s_total - 0.5 * cross_offdiag - diag) / 2.0
    total = 0.5 * pairs_total - m
    return np.float32(total), res


def kernel(topic_embeddings, cluster_ids):
    value, _ = run(topic_embeddings, cluster_ids, trace=False)
    return value
